# revision 1
# baseline (speedup 1.0000x reference)
"""Two-layer GraphSAGE (mean aggregation) on 8 Trainium2 NeuronCores.

Strategy (sharding_hint: shard nodes + edges by destination, replicate
weights, exchange source features for cross-partition edges):

  * Nodes are sharded contiguously across 8 cores (12500 each); edges are
    partitioned by destination shard and sorted into 128-node destination
    windows on the host.
  * Layer-1 aggregation uses linearity: segsum(x[src]) @ W1_l, so raw x rows
    are gathered (x is replicated to every core at upload time - no exchange
    needed for layer 1).
  * Per 128-edge slice, a one-hot matrix M[e,n] = (dst_local[e]==n) is built
    on-device (iota + is_equal) and the segment sum becomes a PE matmul
    accumulated in PSUM: S1[n,f] += M^T @ G.
  * Layer 2 transforms first (P2 = h @ W2_l, 64 cols instead of 128), then
    the P2 shards are exchanged with 4 chunked AllGathers overlapped with
    phase 1, and the second aggregation gathers P2 rows.
  * Rows are fetched with gpsimd dma_gather (int16 indices => the gather
    tables are split into <=32767-row blocks; each window's edges are
    grouped by source block on the host).

Self-contained: hardcodes the problem shapes from the task spec.
"""

import numpy as np

# ---------------------------------------------------------------- config

IN_CH, HIDDEN, OUT_CH = 128, 128, 64
N_NODES, N_EDGES = 100000, 1600000
NCORES = 8
P = 128                      # partitions / window size
L1_RANGE = 25000             # L1 gather block size (int16 limit)


def _derive_cfg(n_nodes):
    shard = n_nodes // NCORES
    nwin = (shard + P - 1) // P
    nchunk = 4 if nwin >= 4 else 1
    chunk_wins = (nwin + nchunk - 1) // nchunk
    # per-core rows per chunk
    chunk_rows = []
    for c in range(nchunk):
        lo = c * chunk_wins * P
        hi = min((c + 1) * chunk_wins * P, shard)
        chunk_rows.append(max(hi - lo, 0))
    ngrp1 = (n_nodes + L1_RANGE - 1) // L1_RANGE
    return dict(shard=shard, nwin=nwin, nchunk=nchunk, chunk_wins=chunk_wins,
                chunk_rows=chunk_rows, ngrp1=ngrp1)


def _round16(x):
    return (x + 15) // 16 * 16


# ---------------------------------------------------------------- host prep

def _preprocess(x, edge_index, cfg):
    n = x.shape[0]
    shard, nwin, nchunk = cfg["shard"], cfg["nwin"], cfg["nchunk"]
    chunk_wins, chunk_rows, ngrp1 = cfg["chunk_wins"], cfg["chunk_rows"], cfg["ngrp1"]

    src = np.asarray(edge_index[0], dtype=np.int64)
    dst = np.asarray(edge_index[1], dtype=np.int64)
    deg = np.bincount(dst, minlength=n).astype(np.float32)
    deg_inv = np.where(deg > 0, np.float32(1.0) / np.maximum(deg, 1.0), 0.0
                       ).astype(np.float32)

    core = dst // shard
    local = dst % shard
    win = local // P
    dstl = (local % P).astype(np.int32)

    # L1 grouping by source range block (block stride L1_RANGE, +1 zero row)
    g1 = np.minimum(src // L1_RANGE, ngrp1 - 1)
    l1loc = (src - g1 * L1_RANGE).astype(np.int32)      # < L1_RANGE+? (last blk)
    l1_blk_rows = [min(L1_RANGE, n - q * L1_RANGE) for q in range(ngrp1)]

    # L2 grouping by source chunk block in the AllGather layout
    csz = chunk_wins * P
    c2 = np.minimum((src % shard) // csz, nchunk - 1)
    # position within block c: (core of src)*chunk_rows[c] + offset in chunk
    l2loc = ((src // shard) * np.array(chunk_rows)[c2]
             + (src % shard) - c2 * csz).astype(np.int32)
    l2_blk_rows = [NCORES * r for r in chunk_rows]      # rows per block (excl zero)

    # static padded counts: max over cores per (win, grp), rounded to 16
    def counts(grp, ngrp):
        key = (core * nwin + win) * ngrp + grp
        cnt = np.bincount(key, minlength=NCORES * nwin * ngrp)
        cnt = cnt.reshape(NCORES, nwin, ngrp)
        return _round16(cnt.max(axis=0))                # [nwin, ngrp]

    T1 = counts(g1, ngrp1)
    T2 = counts(c2, nchunk)
    S1 = np.maximum((T1 + P - 1) // P, 0)               # slices per (win,grp)
    S2 = np.maximum((T2 + P - 1) // P, 0)

    dg_edge = deg_inv[dst]                     # deginv of each edge's dst

    # per-core per-layer packed arrays
    def pack(grp, ngrp, T, S, loc, zero_rows):
        """Build idx16 [128, sumT/16], dstl_f32 and dg_f32 [128, sumS] per core."""
        sumT = int(T.sum())
        sumS = int(S.sum())
        idx_all = np.zeros((NCORES, 16, sumT // 16), np.int16)
        dst_all = np.full((NCORES, P, sumS), 999.0, np.float32)
        dg_all = np.zeros((NCORES, P, sumS), np.float32)
        order = np.lexsort((grp, win, core))
        so, go, wo, co = (a[order] for a in (src, grp, win, core))
        lo_o, dl_o = loc[order], dstl[order]
        dg_o = dg_edge[order]
        # boundaries per (core, win, grp)
        key = (co * nwin + wo) * ngrp + go
        cnt = np.bincount(key, minlength=NCORES * nwin * ngrp
                          ).reshape(NCORES, nwin, ngrp)
        starts = np.zeros_like(cnt)
        pos = 0
        # column offsets of each (win, grp) in the packed arrays (shared)
        colT = np.concatenate([[0], np.cumsum(T.ravel())])[:-1].reshape(T.shape)
        colS = np.concatenate([[0], np.cumsum(S.ravel())])[:-1].reshape(S.shape)
        e0 = 0
        for ci in range(NCORES):
            for w in range(nwin):
                for q in range(ngrp):
                    k = cnt[ci, w, q]
                    ids = lo_o[e0:e0 + k]
                    dls = dl_o[e0:e0 + k]
                    dgs = dg_o[e0:e0 + k]
                    e0 += k
                    t = int(T[w, q])
                    if t == 0:
                        continue
                    buf = np.full(t, zero_rows[q], np.int32)
                    buf[:k] = ids
                    base = int(colT[w, q]) // 16
                    idx_all[ci, :, base:base + t // 16] = (
                        buf.reshape(t // 16, 16).T)
                    db = np.full(((t + P - 1) // P) * P, 999.0, np.float32)
                    db[:k] = dls
                    gb = np.zeros(((t + P - 1) // P) * P, np.float32)
                    gb[:k] = dgs
                    sbase = int(colS[w, q])
                    ns = (t + P - 1) // P
                    dst_all[ci, :, sbase:sbase + ns] = (
                        db.reshape(ns, P).T)
                    dg_all[ci, :, sbase:sbase + ns] = (
                        gb.reshape(ns, P).T)
        assert e0 == len(order)
        # replicate idx 16-partition pattern to 128 partitions
        idx_rep = np.tile(idx_all, (1, 8, 1))
        return idx_rep, dst_all, dg_all, colT, colS, sumT, sumS

    zr1 = l1_blk_rows                          # zero row index per L1 block
    zr2 = l2_blk_rows                          # zero row index per L2 block
    idx1, dst1, dg1, colT1, colS1, sumT1, sumS1 = pack(g1, ngrp1, T1, S1, l1loc, zr1)
    idx2, dst2, dg2, colT2, colS2, sumT2, sumS2 = pack(c2, nchunk, T2, S2, l2loc, zr2)

    # x table with per-block zero row: block q rows [q*(rows+1) ... ]
    xblocks = []
    for q in range(ngrp1):
        xb = x[q * L1_RANGE: q * L1_RANGE + l1_blk_rows[q]]
        xblocks.append(np.concatenate([xb, np.zeros((1, x.shape[1]), np.float32)]))
    xdev = np.concatenate(xblocks, axis=0)
    l1_base = np.concatenate([[0], np.cumsum([b.shape[0] for b in xblocks])])[:-1]

    # per-core transposed shard + deg_inv layout
    xts, dinvs = [], []
    for ci in range(NCORES):
        xs = x[ci * shard:(ci + 1) * shard]
        pad = nwin * P - shard
        xts.append(np.concatenate(
            [xs, np.zeros((pad, x.shape[1]), np.float32)]).T.copy())
        dv = np.concatenate([deg_inv[ci * shard:(ci + 1) * shard],
                             np.zeros(pad, np.float32)])
        dinvs.append(dv.reshape(nwin, P).T.copy())

    meta = dict(T1=T1, T2=T2, S1=S1, S2=S2, colT1=colT1, colS1=colS1,
                colT2=colT2, colS2=colS2, sumT1=sumT1, sumS1=sumS1,
                sumT2=sumT2, sumS2=sumS2, l1_base=l1_base,
                l1_blk_rows=l1_blk_rows, l2_blk_rows=l2_blk_rows)
    data = dict(xdev=xdev, idx1=idx1, dst1=dst1, dg1=dg1, idx2=idx2,
                dst2=dst2, dg2=dg2, xts=xts, dinvs=dinvs)
    return meta, data


# ---------------------------------------------------------------- builder

def _build(cfg, meta, ablate=()):
    import concourse.bacc as bacc
    import concourse.mybir as mybir
    import concourse.tile as tile

    f32 = mybir.dt.float32
    shard, nwin, nchunk = cfg["shard"], cfg["nwin"], cfg["nchunk"]
    chunk_wins, chunk_rows, ngrp1 = cfg["chunk_wins"], cfg["chunk_rows"], cfg["ngrp1"]
    T1, T2, S1, S2 = meta["T1"], meta["T2"], meta["S1"], meta["S2"]
    colT1, colS1 = meta["colT1"], meta["colS1"]
    colT2, colS2 = meta["colT2"], meta["colS2"]
    l1_base = meta["l1_base"]
    l1_blk_rows, l2_blk_rows = meta["l1_blk_rows"], meta["l2_blk_rows"]
    S1w = S1.sum(axis=1)          # slices per window, layer 1
    S2w = S2.sum(axis=1)
    S1max, S2max = int(S1w.max()), int(S2w.max())
    xdev_rows = int(l1_base[-1] + l1_blk_rows[-1] + 1)

    # P2_full block offsets (each block followed by one zero row)
    p2_off = np.concatenate([[0], np.cumsum([r + 1 for r in l2_blk_rows])])
    p2_rows = int(p2_off[-1])

    nc = bacc.Bacc()
    dp = nc.declare_dram_parameter
    xdev = dp("xdev", [xdev_rows, IN_CH], f32, isOutput=False)
    xt = dp("xt", [P, nwin * P], f32, isOutput=False)
    idx1 = dp("idx1", [P, meta["sumT1"] // 16], mybir.dt.int16, isOutput=False)
    dst1 = dp("dst1", [P, meta["sumS1"]], f32, isOutput=False)
    dg1 = dp("dg1", [P, meta["sumS1"]], f32, isOutput=False)
    idx2 = dp("idx2", [P, meta["sumT2"] // 16], mybir.dt.int16, isOutput=False)
    dst2 = dp("dst2", [P, meta["sumS2"]], f32, isOutput=False)
    dg2 = dp("dg2", [P, meta["sumS2"]], f32, isOutput=False)
    w1l = dp("w1l", [IN_CH, HIDDEN], f32, isOutput=False)
    w1r = dp("w1r", [IN_CH, HIDDEN], f32, isOutput=False)
    w2l = dp("w2l", [HIDDEN, OUT_CH], f32, isOutput=False)
    w2r = dp("w2r", [HIDDEN, OUT_CH], f32, isOutput=False)
    b1c = dp("b1c", [P, 1], f32, isOutput=False)
    b2b = dp("b2b", [P, 1], f32, isOutput=False)
    iota = dp("iota", [P, P], f32, isOutput=False)
    ident = dp("ident", [P, P], f32, isOutput=False)
    y = dp("y", [OUT_CH, nwin * P], f32, isOutput=True)

    p2_full = nc.dram_tensor("p2_full", [p2_rows, OUT_CH], f32,
                             addr_space="Shared")

    with tile.TileContext(nc) as tc:
        with (
            tc.tile_pool(name="const", bufs=1) as cb,
            tc.tile_pool(name="sb", bufs=3) as sb,
            tc.tile_pool(name="ps", bufs=2, space="PSUM") as ps,
            tc.tile_pool(name="psb", bufs=1, space="PSUM") as psb,
            tc.tile_pool(name="dram", bufs=1, space="DRAM") as dr,
        ):
            # ---- constants
            def cload(param, shape, tag):
                t = cb.tile(shape, f32, tag=tag)
                nc.sync.dma_start(out=t[:], in_=param[:])
                return t
            iota_t = cload(iota, [P, P], "c_iota")
            ident_t = cload(ident, [P, P], "c_ident")
            w1l_t = cload(w1l, [IN_CH, HIDDEN], "c_w1l")
            w1r_t = cload(w1r, [IN_CH, HIDDEN], "c_w1r")
            w2l_t = cload(w2l, [HIDDEN, OUT_CH], "c_w2l")
            w2r_t = cload(w2r, [HIDDEN, OUT_CH], "c_w2r")
            b1_t = cload(b1c, [P, 1], "c_b1")
            b2_t = cload(b2b, [P, 1], "c_b2")
            r2_t = cb.tile([OUT_CH, nwin * P], f32)     # persistent R2 (transposed)
            zrow_t = cb.tile([P, OUT_CH], f32)
            nc.vector.memset(zrow_t[:], 0.0)

            # P2 chunk DRAM tiles (collective inputs)
            p2c = []
            for c in range(nchunk):
                p2c_tile = dr.tile([max(chunk_rows[c], 1), OUT_CH], f32,
                                   tag=f"p2c{c}")
                p2c.append(p2c_tile)

            # zero rows of p2_full (written once, before collectives run)
            for c in range(nchunk):
                zr = int(p2_off[c] + l2_blk_rows[c])
                nc.sync.dma_start(out=p2_full[zr:zr + 1, :], in_=zrow_t[:1, :])

            relu = mybir.ActivationFunctionType.Relu
            copyf = mybir.ActivationFunctionType.Copy

            # ---------------- phase 1 ----------------
            for w in range(nwin):
                n_w = min(shard - w * P, P)
                s1w = int(S1w[w])
                if s1w == 0:
                    continue
                # load idx/dstl/xt slices for this window
                it = sb.tile([P, int(T1[w].sum()) // 16], mybir.dt.int16, tag="it1")
                nc.sync.dma_start(
                    out=it[:], in_=idx1[:, int(colT1[w, 0]) // 16:
                                        (int(colT1[w, 0]) + int(T1[w].sum())) // 16])
                dt_ = sb.tile([P, s1w], f32, tag="dt1")
                nc.sync.dma_start(
                    out=dt_[:], in_=dst1[:, int(colS1[w, 0]):int(colS1[w, 0]) + s1w])
                dg_ = sb.tile([P, s1w], f32, tag="dg1")
                nc.sync.dma_start(
                    out=dg_[:], in_=dg1[:, int(colS1[w, 0]):int(colS1[w, 0]) + s1w])
                xtw = sb.tile([P, P], f32, tag="xtw")
                nc.sync.dma_start(out=xtw[:], in_=xt[:, w * P:(w + 1) * P])

                # gather slab
                gat = sb.tile([P, S1max * IN_CH], f32, tag="g1")
                nc.vector.memset(gat[:, :s1w * IN_CH], 0.0)
                for q in range(ngrp1):
                    t_q = int(T1[w, q])
                    if t_q == 0:
                        continue
                    cq = (t_q + P - 1) // P
                    sbase = int((S1[w, :q]).sum())
                    ibase = int(colT1[w, q] - colT1[w, 0]) // 16
                    blo = int(l1_base[q])
                    nrows = l1_blk_rows[q] + 1
                    if "nogather" in ablate:
                        continue
                    nc.gpsimd.dma_gather(
                        out_ap=gat[:, sbase * IN_CH:(sbase + cq) * IN_CH]
                        .rearrange("p (c e) -> p c e", e=IN_CH),
                        in_ap=xdev[blo:blo + nrows, :],
                        idxs_ap=it[:, ibase:ibase + t_q // 16],
                        num_idxs=t_q,
                        num_idxs_reg=t_q,
                        elem_size=IN_CH,
                        single_packet=False,
                    )

                # aggregation matmuls: psum1[f,n] += G_g^T @ M_g
                # (one-hot as moving operand; deginv folded into M)
                psum1 = ps.tile([P, IN_CH], f32, tag="ps1", space="PSUM")
                for g in range(s1w):
                    m = sb.tile([P, P], f32, tag="m1")
                    nc.vector.tensor_scalar(
                        out=m[:], in0=iota_t[:], scalar1=dt_[:, g:g + 1],
                        scalar2=dg_[:, g:g + 1],
                        op0=mybir.AluOpType.is_equal,
                        op1=mybir.AluOpType.mult)
                    nc.tensor.matmul(
                        out=psum1[:], lhsT=gat[:, g * IN_CH:(g + 1) * IN_CH],
                        rhs=m[:],
                        start=(g == 0), stop=(g == s1w - 1))

                # T1T = (D S1)^T  [f,n]
                t1t = sb.tile([P, P], f32, tag="t1t")
                nc.vector.tensor_copy(out=t1t[:], in_=psum1[:])
                # hT = relu(W1l^T T1T + W1r^T XTw + b1)  [h,n]
                psum2 = psb.tile([P, P], f32, tag="ps2", space="PSUM")
                nc.tensor.matmul(out=psum2[:], lhsT=w1l_t[:], rhs=t1t[:],
                                 start=True, stop=False)
                nc.tensor.matmul(out=psum2[:], lhsT=w1r_t[:], rhs=xtw[:],
                                 start=False, stop=True)
                ht = sb.tile([P, P], f32, tag="ht")
                nc.vector.tensor_scalar(
                    out=ht[:], in0=psum2[:], scalar1=b1_t[:, :1], scalar2=0.0,
                    op0=mybir.AluOpType.add, op1=mybir.AluOpType.max)
                # DMA-copy hT so it can be a stationary operand (lhsT)
                ht2 = sb.tile([P, P], f32, tag="ht2")
                nc.sync.dma_start(out=ht2[:], in_=ht[:])
                # P2 rows = h @ W2_l  [n,64]
                psum3 = psb.tile([P, OUT_CH], f32, tag="ps3", space="PSUM")
                nc.tensor.matmul(out=psum3[:], lhsT=ht2[:], rhs=w2l_t[:],
                                 start=True, stop=True)
                p2sb = sb.tile([P, OUT_CH], f32, tag="p2sb")
                nc.scalar.activation(out=p2sb[:], in_=psum3[:], func=copyf)
                c = min(w // chunk_wins, nchunk - 1)
                r0 = w * P - c * chunk_wins * P
                nc.sync.dma_start(out=p2c[c][r0:r0 + n_w, :], in_=p2sb[:n_w, :])
                # R2T = (h @ W2_r)^T + b2  [64,n] persistent
                psum4 = psb.tile([OUT_CH, P], f32, tag="ps4", space="PSUM")
                nc.tensor.matmul(out=psum4[:], lhsT=w2r_t[:], rhs=ht[:],
                                 start=True, stop=True)
                nc.vector.tensor_scalar(
                    out=r2_t[:, w * P:(w + 1) * P], in0=psum4[:],
                    scalar1=b2_t[:OUT_CH, :1], scalar2=None,
                    op0=mybir.AluOpType.add)

                # chunk AllGather once its windows are done
                if (w + 1) % chunk_wins == 0 or w == nwin - 1:
                    if (w + 1) % chunk_wins == 0:
                        c_done = (w + 1) // chunk_wins - 1
                    else:
                        c_done = nchunk - 1
                    off = int(p2_off[c_done])
                    rows = l2_blk_rows[c_done]
                    if "noag" in ablate:
                        continue
                    nc.gpsimd.collective_compute(
                        "AllGather",
                        mybir.AluOpType.bypass,
                        replica_groups=[list(range(NCORES))],
                        ins=[p2c[c_done].opt()],
                        outs=[p2_full[off:off + rows, :]],
                    )

            # ---------------- phase 2 ----------------
            for w in range(nwin):
                n_w = min(shard - w * P, P)
                s2w = int(S2w[w])
                if "nophase2" in ablate:
                    s2w = 0
                if s2w == 0:
                    # no edges into this window anywhere: y = R2
                    ysb = sb.tile([OUT_CH, P], f32, tag="ysb")
                    nc.vector.tensor_copy(
                        out=ysb[:], in_=r2_t[:, w * P:(w + 1) * P])
                    nc.sync.dma_start(out=y[:, w * P:(w + 1) * P], in_=ysb[:, :])
                    continue
                it = sb.tile([P, int(T2[w].sum()) // 16], mybir.dt.int16, tag="it2")
                nc.sync.dma_start(
                    out=it[:], in_=idx2[:, int(colT2[w, 0]) // 16:
                                        (int(colT2[w, 0]) + int(T2[w].sum())) // 16])
                dt_ = sb.tile([P, s2w], f32, tag="dt2")
                nc.sync.dma_start(
                    out=dt_[:], in_=dst2[:, int(colS2[w, 0]):int(colS2[w, 0]) + s2w])
                dg_ = sb.tile([P, s2w], f32, tag="dg2")
                nc.sync.dma_start(
                    out=dg_[:], in_=dg2[:, int(colS2[w, 0]):int(colS2[w, 0]) + s2w])
                gat = sb.tile([P, S2max * OUT_CH], f32, tag="g2")
                nc.vector.memset(gat[:, :s2w * OUT_CH], 0.0)
                for q in range(nchunk):
                    t_q = int(T2[w, q])
                    if t_q == 0:
                        continue
                    cq = (t_q + P - 1) // P
                    sbase = int((S2[w, :q]).sum())
                    ibase = int(colT2[w, q] - colT2[w, 0]) // 16
                    off = int(p2_off[q])
                    nrows = l2_blk_rows[q] + 1
                    if "nogather" in ablate:
                        continue
                    nc.gpsimd.dma_gather(
                        out_ap=gat[:, sbase * OUT_CH:(sbase + cq) * OUT_CH]
                        .rearrange("p (c e) -> p c e", e=OUT_CH),
                        in_ap=p2_full[off:off + nrows, :],
                        idxs_ap=it[:, ibase:ibase + t_q // 16],
                        num_idxs=t_q,
                        num_idxs_reg=t_q,
                        elem_size=OUT_CH,
                        single_packet=False,
                    )
                psum5 = ps.tile([OUT_CH, P], f32, tag="ps5", space="PSUM")
                for g in range(s2w):
                    m = sb.tile([P, P], f32, tag="m2")
                    nc.vector.tensor_scalar(
                        out=m[:], in0=iota_t[:], scalar1=dt_[:, g:g + 1],
                        scalar2=dg_[:, g:g + 1],
                        op0=mybir.AluOpType.is_equal,
                        op1=mybir.AluOpType.mult)
                    nc.tensor.matmul(
                        out=psum5[:], lhsT=gat[:, g * OUT_CH:(g + 1) * OUT_CH],
                        rhs=m[:],
                        start=(g == 0), stop=(g == s2w - 1))
                ysb = sb.tile([OUT_CH, P], f32, tag="ysb")
                nc.vector.tensor_add(out=ysb[:], in0=psum5[:],
                                     in1=r2_t[:, w * P:(w + 1) * P])
                nc.sync.dma_start(out=y[:, w * P:(w + 1) * P], in_=ysb[:, :])

    nc.compile()
    return nc


# ---------------------------------------------------------------- entry

_CACHE = {}


def kernel(x, edge_index, W1_l, W1_r, b1, W2_l, W2_r, b2):
    x = np.asarray(x, dtype=np.float32)
    edge_index = np.asarray(edge_index)
    cfg = _derive_cfg(x.shape[0])
    meta, data = _preprocess(x, edge_index, cfg)

    key = (x.shape, edge_index.shape)
    if key in _CACHE and _CACHE[key][1] == _meta_sig(meta):
        nc = _CACHE[key][0]
    else:
        nc = _build(cfg, meta)
        _CACHE[key] = (nc, _meta_sig(meta))

    in_maps = _make_inmaps(
        dict(W1_l=W1_l, W1_r=W1_r, b1=b1, W2_l=W2_l, W2_r=W2_r, b2=b2),
        meta, data)

    from concourse.bass_utils import run_bass_kernel_spmd
    r = run_bass_kernel_spmd(nc, in_maps, core_ids=list(range(NCORES)))
    shard = cfg["shard"]
    out = np.concatenate(
        [r.results[c]["y"].T[:shard] for c in range(NCORES)], axis=0)
    return np.ascontiguousarray(out, dtype=np.float32)


def _meta_sig(meta):
    return (int(meta["sumT1"]), int(meta["sumS1"]),
            int(meta["sumT2"]), int(meta["sumS2"]))


def _make_inmaps(inputs, meta, data):
    iota_v = np.tile(np.arange(P, dtype=np.float32), (P, 1))
    ident_v = np.eye(P, dtype=np.float32)
    common = dict(
        xdev=data["xdev"],
        w1l=np.asarray(inputs["W1_l"], np.float32),
        w1r=np.asarray(inputs["W1_r"], np.float32),
        w2l=np.asarray(inputs["W2_l"], np.float32),
        w2r=np.asarray(inputs["W2_r"], np.float32),
        b1c=np.asarray(inputs["b1"], np.float32).reshape(P, 1),
        b2b=np.concatenate([np.asarray(inputs["b2"], np.float32),
                            np.zeros(P - OUT_CH, np.float32)]).reshape(P, 1),
        iota=iota_v, ident=ident_v,
    )
    in_maps = []
    for ci in range(NCORES):
        m = dict(common)
        m["xt"] = data["xts"][ci]
        m["idx1"] = data["idx1"][ci]
        m["dst1"] = data["dst1"][ci]
        m["dg1"] = data["dg1"][ci]
        m["idx2"] = data["idx2"][ci]
        m["dst2"] = data["dst2"][ci]
        m["dg2"] = data["dg2"][ci]
        in_maps.append(m)
    return in_maps



# revision 3
# speedup vs baseline: 1.3591x; 1.3591x over previous
"""Two-layer GraphSAGE (mean agg) on 8 Trainium2 cores — low-instruction-count
design (v2).

Per-core (nodes sharded by dst, weights replicated):
  * Shard nodes permuted by in-degree asc; windows of 128 grouped into
    groups of <=4 windows with shared (max-over-cores) padded degree D.
  * Aggregation per group: stage-1 = <=4 block dma_gathers (int16 idx) of
    edge source rows into SBUF -> 1 DMA to a DRAM staging table; stage-2 =
    one transpose-mode dma_gather producing a [128f, Gn*D] bf16 slab in
    node-major neighbor order; one tensor_reduce(add) over D = segment sum;
    one broadcast-multiply applies deg_inv.
  * Transforms batched per group (<=512 cols per psum); bf16 throughout.
  * x AllGathered on device from per-core shard uploads; P2 AllGathered in
    chunks overlapped with phase A. Output y [64, npad] per core, host
    un-permutes.
"""

import numpy as np

IN_CH, HIDDEN, OUT_CH = 128, 128, 64
N_NODES, N_EDGES = 100000, 1600000
NCORES = 8
P = 128
CAP_EDGES = 10240          # max padded edges (Gn*D) per group
MAX_GW = 4                 # max windows per group
BLK_TARGET = 30000         # max rows per int16 gather block


def _derive_cfg(n_nodes):
    shard = n_nodes // NCORES
    nwin = (shard + P - 1) // P
    npad = nwin * P
    nchunk = 2 if nwin >= 8 else 1
    cw = (nwin + nchunk - 1) // nchunk
    chunk_wins = [min(cw, nwin - c * cw) for c in range(nchunk)]
    xrows = n_nodes + 1                      # + zero row
    nblk1 = max(1, -(-xrows // BLK_TARGET))
    p2_rows = NCORES * npad + 1              # + zero row
    nblk2 = max(1, -(-p2_rows // BLK_TARGET))
    return dict(shard=shard, nwin=nwin, npad=npad, nchunk=nchunk, cw=cw,
                chunk_wins=chunk_wins, xrows=xrows, nblk1=nblk1,
                p2_rows=p2_rows, nblk2=nblk2)


def _blk_bounds(total, nblk):
    return [round(i * total / nblk) for i in range(nblk + 1)]


def _wrap16(vals):
    v = np.asarray(vals, np.int16)
    return v.reshape(-1, 16).T.copy()


def _preprocess(x, edge_index, cfg):
    n = x.shape[0]
    S, nwin, npad = cfg["shard"], cfg["nwin"], cfg["npad"]
    nchunk, cw = cfg["nchunk"], cfg["cw"]
    xrows, nblk1 = cfg["xrows"], cfg["nblk1"]
    p2_rows, nblk2 = cfg["p2_rows"], cfg["nblk2"]
    b1b = _blk_bounds(xrows, nblk1)
    b2b = _blk_bounds(p2_rows, nblk2)

    src = np.asarray(edge_index[0], dtype=np.int64)
    dst = np.asarray(edge_index[1], dtype=np.int64)
    deg = np.bincount(dst, minlength=n).astype(np.int64)
    deg_inv = np.where(deg > 0, 1.0 / np.maximum(deg, 1), 0.0).astype(np.float32)

    core_of = dst // S
    perms = []
    pos_of = np.zeros(n, np.int64)
    degs_pad_c = []
    for c in range(NCORES):
        dc = np.concatenate([deg[c * S:(c + 1) * S],
                             np.full(npad - S, -1, np.int64)])
        perm = np.argsort(dc, kind="stable")
        perms.append(perm)
        inv = np.empty(npad, np.int64)
        inv[perm] = np.arange(npad)
        pos_of[c * S:(c + 1) * S] = inv[:S]
        degs_pad_c.append(np.maximum(dc[perm], 0))

    rows_k = [w * P for w in cfg["chunk_wins"]]
    chunk_base = np.concatenate(
        [[0], np.cumsum([NCORES * r for r in rows_k])]).astype(np.int64)
    node_core = np.arange(n, dtype=np.int64) // S
    k_of = np.minimum(pos_of // (cw * P), nchunk - 1)
    p2row = (chunk_base[k_of] + node_core * np.array(rows_k)[k_of]
             + (pos_of - k_of * cw * P)).astype(np.int64)

    # shared window max degree
    dwin = np.zeros(nwin, np.int64)
    for c in range(NCORES):
        dwin = np.maximum(dwin, degs_pad_c[c].reshape(nwin, P).max(axis=1))
    dwin = np.maximum(dwin, 1)

    # shared greedy groups within chunk boundaries
    groups = []
    for k in range(nchunk):
        w0k, w1k = k * cw, min((k + 1) * cw, nwin)
        w = w0k
        while w < w1k:
            gw, D = 1, int(dwin[w])
            for g2 in range(2, MAX_GW + 1):
                if w + g2 > w1k:
                    break
                d2 = int(dwin[w:w + g2].max())
                if g2 * P * d2 <= CAP_EDGES:
                    gw, D = g2, d2
                else:
                    break
            groups.append((w, gw, D, k))
            w += gw

    # per-core edge arrays sorted by (core, pos, src)
    pos_of_dst = pos_of[dst]
    order = np.lexsort((src, pos_of_dst, core_of))
    e_core, e_pos, e_src = core_of[order], pos_of_dst[order], src[order]
    core_sl = [slice(*np.searchsorted(e_core, [c, c + 1]))
               for c in range(NCORES)]

    # per (core, group): edge slices + per-block rowids
    def stage_tables(bounds, nblk, zero_row, rowid_fn):
        """Returns shared call meta + per-core wrapped idx + per-core grids."""
        # counts per (group, core, block)
        cnt = np.zeros((len(groups), NCORES, nblk), np.int64)
        blk_e = []   # per core: block id per edge
        rid_e = []   # per core: rowid per edge
        for c in range(NCORES):
            cpos = e_pos[core_sl[c]]
            rid = rowid_fn(c)
            blk = np.minimum(np.searchsorted(bounds, rid, side="right") - 1,
                             nblk - 1)
            blk_e.append(blk)
            rid_e.append(rid)
            for gi, (w0, gw, D, k) in enumerate(groups):
                lo, hi = np.searchsorted(cpos, [w0 * P, (w0 + gw) * P])
                cnt[gi, c] += np.bincount(blk[lo:hi], minlength=nblk)
        cnt[:, :, nblk - 1] += 1          # zero entry
        tq = cnt.max(axis=1)              # [ngroups, nblk]
        tpad = -(-tq // 128) * 128
        bases = np.concatenate(
            [np.zeros((len(groups), 1), np.int64), np.cumsum(tpad, axis=1)],
            axis=1)
        nstage = bases[:, -1]
        calls = []   # per group: list of (q, num_idxs, col_off16, b0, b1)
        coloff = 0
        colgrp = []
        for gi in range(len(groups)):
            cl = []
            for q in range(nblk):
                if tpad[gi, q]:
                    cl.append((q, int(tpad[gi, q]), coloff,
                               bounds[q], bounds[q + 1]))
                    coloff += int(tpad[gi, q]) // 16
            calls.append(cl)
            colgrp.append(coloff)
        total16 = coloff

        idx_cores = []
        grids = []
        for c in range(NCORES):
            cpos = e_pos[core_sl[c]]
            blk = blk_e[c]
            rid = rid_e[c]
            wr = np.full((16, total16), -1, np.int16)
            grid_list = []
            for gi, (w0, gw, D, k) in enumerate(groups):
                Gn = gw * P
                lo, hi = np.searchsorted(cpos, [w0 * P, (w0 + gw) * P])
                gpos = cpos[lo:hi] - w0 * P
                gblk = blk[lo:hi]
                grid = rid[lo:hi]
                slots = np.zeros(hi - lo, np.int64)
                zero_slot = int(bases[gi, nblk - 1])
                for (q, tp, co16, b0, b1) in calls[gi]:
                    m = gblk == q
                    ids = grid[m] - b0
                    if q == nblk - 1:
                        ids = np.concatenate([[zero_row - b0], ids])
                    t = len(ids)
                    buf = np.zeros(tp, np.int64)
                    buf[:t] = ids
                    assert t <= tp and (t == 0 or (ids.max() < 32768
                                                   and ids.min() >= 0))
                    wr[:, co16:co16 + tp // 16] = _wrap16(buf)
                    base = int(bases[gi, q])
                    if q == nblk - 1:
                        slots[m] = base + 1 + np.arange(t - 1)
                    else:
                        slots[m] = base + np.arange(t)
                # node-major (pos, j) grid
                jj = np.arange(hi - lo) - np.concatenate(
                    [[0], np.cumsum(np.bincount(gpos, minlength=Gn))])[gpos]
                gridm = np.full((Gn, D), zero_slot, np.int64)
                gridm[gpos, jj] = slots
                assert gridm.max() < 32768
                grid_list.append(_wrap16(gridm.ravel()))
            idx_cores.append(wr)
            grids.append(np.concatenate(grid_list, axis=1))
        grid_off = np.concatenate(
            [[0], np.cumsum([(g[1] * P * g[2]) // 16 for g in groups])])
        return dict(calls=calls, nstage=nstage, idx=idx_cores, grids=grids,
                    grid_off=grid_off, total16=total16)

    st1 = stage_tables(b1b, nblk1, xrows - 1,
                       lambda c: e_src[core_sl[c]])
    st2 = stage_tables(b2b, nblk2, p2_rows - 1,
                       lambda c: p2row[e_src[core_sl[c]]])

    cores = []
    for c in range(NCORES):
        perm = perms[c]
        real = perm < S
        xp = np.zeros((npad, IN_CH), np.float32)
        xp[real] = x[c * S + perm[real]]
        dinvp = np.zeros(npad, np.float32)
        dinvp[real] = deg_inv[c * S + perm[real]]
        cores.append(dict(xt=xp.T.copy(), dinv=dinvp.reshape(1, npad),
                          xsh=x[c * S:(c + 1) * S], perm=perm))

    shared = dict(groups=groups, st1calls=st1["calls"], st1n=st1["nstage"],
                  st1w=st1["total16"], st2calls=st2["calls"],
                  st2n=st2["nstage"], st2w=st2["total16"],
                  grid_off=st1["grid_off"], chunk_base=chunk_base,
                  rows_k=rows_k)
    data = dict(cores=cores, g1=st1["idx"], g2=st1["grids"],
                h1=st2["idx"], h2=st2["grids"])
    return shared, data


# ---------------------------------------------------------------- builder

def _build(cfg, shared):
    import concourse.bacc as bacc
    import concourse.mybir as mybir
    import concourse.tile as tile

    f32 = mybir.dt.float32
    bf16 = mybir.dt.bfloat16
    i16 = mybir.dt.int16

    S, nwin, npad = cfg["shard"], cfg["nwin"], cfg["npad"]
    nchunk, cw = cfg["nchunk"], cfg["cw"]
    xrows, p2_rows = cfg["xrows"], cfg["p2_rows"]
    groups = shared["groups"]
    st1calls, st1n = shared["st1calls"], shared["st1n"]
    st2calls, st2n = shared["st2calls"], shared["st2n"]
    grid_off = shared["grid_off"]
    chunk_base, rows_k = shared["chunk_base"], shared["rows_k"]
    maxst = int(max(st1n.max(), st2n.max()))
    maxGnD = max(g[1] * P * g[2] for g in groups)
    maxGn = max(g[1] for g in groups) * P

    nc = bacc.Bacc()
    dp = nc.declare_dram_parameter
    xsh = dp("xsh", [S, IN_CH], bf16, isOutput=False)
    xtp = dp("xtp", [P, npad], bf16, isOutput=False)
    dinv = dp("dinv", [1, npad], f32, isOutput=False)
    g1p = dp("g1p", [16, shared["st1w"]], i16, isOutput=False)
    g2p = dp("g2p", [16, int(grid_off[-1])], i16, isOutput=False)
    h1p = dp("h1p", [16, shared["st2w"]], i16, isOutput=False)
    h2p = dp("h2p", [16, int(grid_off[-1])], i16, isOutput=False)
    w1l = dp("w1l", [IN_CH, HIDDEN], bf16, isOutput=False)
    w1r = dp("w1r", [IN_CH, HIDDEN], bf16, isOutput=False)
    w2l = dp("w2l", [HIDDEN, OUT_CH], bf16, isOutput=False)
    w2r = dp("w2r", [HIDDEN, OUT_CH], bf16, isOutput=False)
    b1c = dp("b1c", [P, 1], f32, isOutput=False)
    b2c = dp("b2c", [OUT_CH, 1], f32, isOutput=False)
    y = dp("y", [OUT_CH, npad], f32, isOutput=True)

    xdev = nc.dram_tensor("xdev", [xrows, IN_CH], bf16, addr_space="Shared")
    p2f = nc.dram_tensor("p2f", [p2_rows, P], bf16, addr_space="Shared")
    # 128-replicated idx tables (built on device once)
    g1r = nc.dram_tensor("g1r", [P, shared["st1w"]], i16)
    g2r = nc.dram_tensor("g2r", [P, int(grid_off[-1])], i16)
    h1r = nc.dram_tensor("h1r", [P, shared["st2w"]], i16)
    h2r = nc.dram_tensor("h2r", [P, int(grid_off[-1])], i16)

    with tile.TileContext(nc) as tc:
        with (
            tc.tile_pool(name="const", bufs=1) as cb,
            tc.tile_pool(name="st", bufs=1) as stp,
            tc.tile_pool(name="sl", bufs=2) as slp,
            tc.tile_pool(name="sb", bufs=2) as sb,
            tc.tile_pool(name="ix", bufs=2) as ixp,
            tc.tile_pool(name="psa", bufs=2, space="PSUM") as psa,
            tc.tile_pool(name="psb", bufs=2, space="PSUM") as psb,
            tc.tile_pool(name="psr", bufs=1, space="PSUM") as psr,
            tc.tile_pool(name="psd", bufs=2, space="PSUM") as psd,
            tc.tile_pool(name="dram", bufs=2, space="DRAM") as dr,
        ):
            def cload(param, shape, dt, tag):
                t = cb.tile(shape, dt, tag=tag)
                nc.sync.dma_start(out=t[:], in_=param[:])
                return t

            w1l_t = cload(w1l, [P, HIDDEN], bf16, "w1l")
            w1r_t = cload(w1r, [P, HIDDEN], bf16, "w1r")
            w2l_t = cload(w2l, [P, OUT_CH], bf16, "w2l")
            w2r_t = cload(w2r, [P, OUT_CH], bf16, "w2r")
            b1_t = cload(b1c, [P, 1], f32, "b1")
            b2_t = cload(b2c, [OUT_CH, 1], f32, "b2")
            xt_t = cload(xtp, [P, npad], bf16, "xt")
            zrow = cb.tile([P, P], bf16, tag="zrow")
            nc.vector.memset(zrow[:], 0.0)
            ones_t = cb.tile([1, P], f32, tag="ones")
            nc.vector.memset(ones_t[:], 1.0)
            r2_t = cb.tile([OUT_CH, npad], f32, tag="r2")

            # replicate idx tables [16, W] -> [128, W] in DRAM
            for par, rep in ((g1p, g1r), (g2p, g2r), (h1p, h1r), (h2p, h2r)):
                for rr in range(8):
                    nc.sync.dma_start(out=rep[rr * 16:(rr + 1) * 16, :],
                                      in_=par[:])

            # zero rows
            nc.sync.dma_start(out=xdev[xrows - 1:xrows, :], in_=zrow[:1, :])
            nc.sync.dma_start(out=p2f[p2_rows - 1:p2_rows, :], in_=zrow[:1, :])

            # x AllGather
            xc = dr.tile([S, IN_CH], bf16, tag="xc")
            nc.sync.dma_start(out=xc[:], in_=xsh[:])
            nc.gpsimd.collective_compute(
                "AllGather", mybir.AluOpType.bypass,
                replica_groups=[list(range(NCORES))],
                ins=[xc.opt()],
                outs=[xdev[0:NCORES * S, :]],
            )

            p2c = []
            for k in range(nchunk):
                p2ck = dr.tile([rows_k[k], P], bf16, tag=f"p2c{k}")
                p2c.append(p2ck)

            def do_stage(gi, calls, nst, idx_rep, table_ap, tag):
                """stage-1 gathers + DMA to DRAM staging; returns dram tile."""
                n = int(nst[gi])
                stt = stp.tile([P, (maxst // P) * IN_CH], bf16, tag="st")
                c0 = calls[gi][0][2]
                cn = sum(num for (_, num, _, _, _) in calls[gi]) // 16
                it = ixp.tile([P, maxst // 16], i16, tag="it")
                nc.sync.dma_start(out=it[:, 0:cn],
                                  in_=idx_rep[:, c0:c0 + cn])
                base = 0
                for (q, num, co16, b0, b1_) in calls[gi]:
                    nc.gpsimd.dma_gather(
                        out_ap=stt[:, (base // P) * IN_CH:
                                   ((base + num) // P) * IN_CH]
                        .rearrange("p (c e) -> p c e", e=IN_CH),
                        in_ap=table_ap(b0, b1_),
                        idxs_ap=it[:, co16 - c0:co16 - c0 + num // 16],
                        num_idxs=num, num_idxs_reg=num,
                        elem_size=IN_CH, single_packet=False)
                    base += num
                sd = dr.tile([maxst, IN_CH], bf16, tag="sd")
                nc.sync.dma_start(
                    out=sd[0:n, :].rearrange("(c p) e -> p c e", p=P),
                    in_=stt[:, 0:(n // P) * IN_CH].rearrange(
                        "p (c e) -> p c e", e=IN_CH))
                return sd, n

            def do_slab(gi, sd, n, grid_rep, tag):
                w0, gw, D, k = groups[gi]
                Gn = gw * P
                it = ixp.tile([P, (Gn * D) // 16], i16, tag="gg")
                nc.sync.dma_start(
                    out=it[:],
                    in_=grid_rep[:, int(grid_off[gi]):int(grid_off[gi + 1])])
                slab = slp.tile([P, maxGnD], bf16, tag="sl")
                nc.gpsimd.dma_gather(
                    out_ap=slab[:, 0:Gn * D].rearrange(
                        "p (c e) -> p c e", c=1),
                    in_ap=sd[0:n, :],
                    idxs_ap=it[:],
                    num_idxs=Gn * D, num_idxs_reg=Gn * D,
                    elem_size=IN_CH, transpose=True, single_packet=False)
                return slab

            # ---------------- phase A ----------------
            for gi, (w0, gw, D, k) in enumerate(groups):
                Gn = gw * P
                p0 = w0 * P
                sd, n1 = do_stage(gi, st1calls, st1n, g1r,
                                  lambda b0, b1_: xdev[b0:b1_, :], "1")
                slab = do_slab(gi, sd, n1, g2r, "1")
                dv = ixp.tile([1, maxGn], f32, tag="dv")
                nc.sync.dma_start(out=dv[0:1, 0:Gn],
                                  in_=dinv[0:1, p0:p0 + Gn])
                agg = sb.tile([P, maxGn], f32, tag="agg")
                nc.vector.tensor_reduce(
                    out=agg[:, 0:Gn],
                    in_=slab[:, 0:Gn * D].rearrange("p (n d) -> p n d", d=D),
                    axis=mybir.AxisListType.X, op=mybir.AluOpType.add)
                psdv = psd.tile([P, maxGn], f32, tag="psdv", space="PSUM")
                nc.tensor.matmul(out=psdv[:, 0:Gn], lhsT=ones_t[0:1, :],
                                 rhs=dv[0:1, 0:Gn], start=True, stop=True)
                aggs = sb.tile([P, maxGn], bf16, tag="aggs")
                nc.vector.tensor_tensor(
                    out=aggs[:, 0:Gn], in0=agg[:, 0:Gn],
                    in1=psdv[:, 0:Gn], op=mybir.AluOpType.mult)
                ps1 = psa.tile([P, maxGn], f32, tag="ps1", space="PSUM")
                nc.tensor.matmul(out=ps1[:, 0:Gn], lhsT=w1l_t[:],
                                 rhs=aggs[:, 0:Gn], start=True, stop=False)
                nc.tensor.matmul(out=ps1[:, 0:Gn], lhsT=w1r_t[:],
                                 rhs=xt_t[:, p0:p0 + Gn], start=False,
                                 stop=True)
                h4 = sb.tile([P, maxGn], bf16, tag="h4")
                nc.vector.tensor_scalar(
                    out=h4[:, 0:Gn], in0=ps1[:, 0:Gn], scalar1=b1_t[:, 0:1],
                    scalar2=0.0, op0=mybir.AluOpType.add,
                    op1=mybir.AluOpType.max)
                h4c = sb.tile([P, maxGn], bf16, tag="h4c")
                nc.vector.tensor_copy(out=h4c[:, 0:Gn], in_=h4[:, 0:Gn])
                # P2 rows (node-major) for this group's windows
                ps2 = psb.tile([P, MAX_GW * OUT_CH], f32, tag="ps2",
                               space="PSUM")
                for wi in range(gw):
                    nc.tensor.matmul(
                        out=ps2[:, wi * OUT_CH:(wi + 1) * OUT_CH],
                        lhsT=h4c[:, wi * P:(wi + 1) * P], rhs=w2l_t[:],
                        start=True, stop=True)
                p2sb = sb.tile([P, MAX_GW * P], bf16, tag="p2sb")
                nc.vector.memset(p2sb[:, 0:gw * P], 0.0)
                nc.vector.tensor_copy(
                    out=p2sb[:, 0:gw * P].rearrange(
                        "p (w c) -> p w c", c=P)[:, :, 0:OUT_CH],
                    in_=ps2[:, 0:gw * OUT_CH].rearrange(
                        "p (w c) -> p w c", c=OUT_CH))
                q0 = (w0 - k * cw) * P
                nc.sync.dma_start(
                    out=p2c[k][q0:q0 + Gn, :].rearrange(
                        "(w l) c -> l w c", l=P),
                    in_=p2sb[:, 0:gw * P].rearrange(
                        "l (w c) -> l w c", c=P))
                # R2 = h @ W2r + b2 (transposed), kept in SBUF
                ps3 = psr.tile([OUT_CH, maxGn], f32, tag="ps3", space="PSUM")
                nc.tensor.matmul(out=ps3[:, 0:Gn], lhsT=w2r_t[:],
                                 rhs=h4[:, 0:Gn], start=True, stop=True)
                nc.vector.tensor_scalar(
                    out=r2_t[:, p0:p0 + Gn], in0=ps3[:, 0:Gn],
                    scalar1=b2_t[:, 0:1], scalar2=None,
                    op0=mybir.AluOpType.add)

                if gi + 1 == len(groups) or groups[gi + 1][3] != k:
                    nc.gpsimd.collective_compute(
                        "AllGather", mybir.AluOpType.bypass,
                        replica_groups=[list(range(NCORES))],
                        ins=[p2c[k].opt()],
                        outs=[p2f[int(chunk_base[k]):int(chunk_base[k + 1]),
                                  :]],
                    )

            # ---------------- phase B ----------------
            for gi, (w0, gw, D, k) in enumerate(groups):
                Gn = gw * P
                p0 = w0 * P
                sd, n2 = do_stage(gi, st2calls, st2n, h1r,
                                  lambda b0, b1_: p2f[b0:b1_, :], "2")
                slab = do_slab(gi, sd, n2, h2r, "2")
                dv = ixp.tile([1, maxGn], f32, tag="dv")
                nc.sync.dma_start(out=dv[0:1, 0:Gn],
                                  in_=dinv[0:1, p0:p0 + Gn])
                agg = sb.tile([P, maxGn], f32, tag="agg")
                nc.vector.tensor_reduce(
                    out=agg[:, 0:Gn],
                    in_=slab[:, 0:Gn * D].rearrange("p (n d) -> p n d", d=D),
                    axis=mybir.AxisListType.X, op=mybir.AluOpType.add)
                psdv = psd.tile([P, maxGn], f32, tag="psdv", space="PSUM")
                nc.tensor.matmul(out=psdv[:, 0:Gn], lhsT=ones_t[0:1, :],
                                 rhs=dv[0:1, 0:Gn], start=True, stop=True)
                ym = sb.tile([OUT_CH, maxGn], f32, tag="ym")
                nc.vector.tensor_tensor(
                    out=ym[:, 0:Gn], in0=agg[0:OUT_CH, 0:Gn],
                    in1=psdv[0:OUT_CH, 0:Gn], op=mybir.AluOpType.mult)
                yo = sb.tile([OUT_CH, maxGn], f32, tag="yo")
                nc.vector.tensor_tensor(
                    out=yo[:, 0:Gn], in0=ym[:, 0:Gn],
                    in1=r2_t[:, p0:p0 + Gn], op=mybir.AluOpType.add)
                nc.sync.dma_start(out=y[:, p0:p0 + Gn], in_=yo[:, 0:Gn])

    nc.compile()
    return nc


# ---------------------------------------------------------------- entry

_CACHE = {}


def _to_bf16(a):
    import ml_dtypes
    return np.asarray(a, np.float32).astype(ml_dtypes.bfloat16)


def _make_inmaps(inputs, shared, data):
    common = dict(
        w1l=_to_bf16(inputs["W1_l"]),
        w1r=_to_bf16(inputs["W1_r"]),
        w2l=_to_bf16(inputs["W2_l"]),
        w2r=_to_bf16(inputs["W2_r"]),
        b1c=np.asarray(inputs["b1"], np.float32).reshape(P, 1),
        b2c=np.asarray(inputs["b2"], np.float32).reshape(OUT_CH, 1),
    )
    in_maps = []
    for c in range(NCORES):
        d = data["cores"][c]
        m = dict(common)
        m["xsh"] = _to_bf16(d["xsh"])
        m["xtp"] = _to_bf16(d["xt"])
        m["dinv"] = d["dinv"]
        m["g1p"] = data["g1"][c]
        m["g2p"] = data["g2"][c]
        m["h1p"] = data["h1"][c]
        m["h2p"] = data["h2"][c]
        in_maps.append(m)
    return in_maps


def kernel(x, edge_index, W1_l, W1_r, b1, W2_l, W2_r, b2):
    x = np.asarray(x, dtype=np.float32)
    edge_index = np.asarray(edge_index)
    cfg = _derive_cfg(x.shape[0])
    shared, data = _preprocess(x, edge_index, cfg)

    key = (x.shape, edge_index.shape)
    sig = _sig_of(shared)
    if key in _CACHE and _CACHE[key][1] == sig:
        nc = _CACHE[key][0]
    else:
        nc = _build(cfg, shared)
        _CACHE[key] = (nc, sig)

    in_maps = _make_inmaps(
        dict(W1_l=W1_l, W1_r=W1_r, b1=b1, W2_l=W2_l, W2_r=W2_r, b2=b2),
        shared, data)

    from concourse.bass_utils import run_bass_kernel_spmd
    r = run_bass_kernel_spmd(nc, in_maps, core_ids=list(range(NCORES)))
    S = cfg["shard"]
    out = np.zeros((x.shape[0], OUT_CH), np.float32)
    for c in range(NCORES):
        yc = np.asarray(r.results[c]["y"], np.float32).T  # [npad, 64]
        perm = data["cores"][c]["perm"]
        real = perm < S
        out[c * S + perm[real]] = yc[real]
    return np.ascontiguousarray(out, dtype=np.float32)


def _sig_of(shared):
    return (tuple(shared["groups"]), int(shared["st1w"]),
            int(shared["st2w"]), int(shared["grid_off"][-1]))


# revision 5
# speedup vs baseline: 1.7265x; 1.2703x over previous
"""Two-layer GraphSAGE (mean agg) on 8 Trainium2 cores — low-instruction-count
design (v2).

Per-core (nodes sharded by dst, weights replicated):
  * Shard nodes permuted by in-degree asc; windows of 128 grouped into
    groups of <=4 windows with shared (max-over-cores) padded degree D.
  * Aggregation per group: stage-1 = <=4 block dma_gathers (int16 idx) of
    edge source rows into SBUF -> 1 DMA to a DRAM staging table; stage-2 =
    one transpose-mode dma_gather producing a [128f, Gn*D] bf16 slab in
    node-major neighbor order; one tensor_reduce(add) over D = segment sum;
    one broadcast-multiply applies deg_inv.
  * Transforms batched per group (<=512 cols per psum); bf16 throughout.
  * x AllGathered on device from per-core shard uploads; P2 AllGathered in
    chunks overlapped with phase A. Output y [64, npad] per core, host
    un-permutes.
"""

import numpy as np

IN_CH, HIDDEN, OUT_CH = 128, 128, 64
N_NODES, N_EDGES = 100000, 1600000
NCORES = 8
P = 128
CAP_EDGES = 10240          # max padded edges (Gn*D) per group
MAX_GW = 4                 # max windows per group
BLK_TARGET = 30000         # max rows per int16 gather block


def _derive_cfg(n_nodes):
    shard = n_nodes // NCORES
    nwin = (shard + P - 1) // P
    npad = nwin * P
    nchunk = 2 if nwin >= 8 else 1
    cw = (nwin + nchunk - 1) // nchunk
    chunk_wins = [min(cw, nwin - c * cw) for c in range(nchunk)]
    xrows = n_nodes + 1                      # + zero row
    nblk1 = max(1, -(-xrows // BLK_TARGET))
    p2_rows = NCORES * npad + 1              # + zero row
    nblk2 = max(1, -(-p2_rows // BLK_TARGET))
    return dict(shard=shard, nwin=nwin, npad=npad, nchunk=nchunk, cw=cw,
                chunk_wins=chunk_wins, xrows=xrows, nblk1=nblk1,
                p2_rows=p2_rows, nblk2=nblk2)


def _blk_bounds(total, nblk):
    return [round(i * total / nblk) for i in range(nblk + 1)]


def _wrap16(vals):
    v = np.asarray(vals, np.int16)
    return v.reshape(-1, 16).T.copy()


def _preprocess(x, edge_index, cfg):
    n = x.shape[0]
    S, nwin, npad = cfg["shard"], cfg["nwin"], cfg["npad"]
    nchunk, cw = cfg["nchunk"], cfg["cw"]
    xrows, nblk1 = cfg["xrows"], cfg["nblk1"]
    p2_rows, nblk2 = cfg["p2_rows"], cfg["nblk2"]
    b1b = _blk_bounds(xrows, nblk1)
    b2b = _blk_bounds(p2_rows, nblk2)

    src = np.asarray(edge_index[0], dtype=np.int64)
    dst = np.asarray(edge_index[1], dtype=np.int64)
    deg = np.bincount(dst, minlength=n).astype(np.int64)
    deg_inv = np.where(deg > 0, 1.0 / np.maximum(deg, 1), 0.0).astype(np.float32)

    core_of = dst // S
    perms = []
    pos_of = np.zeros(n, np.int64)
    degs_pad_c = []
    for c in range(NCORES):
        dc = np.concatenate([deg[c * S:(c + 1) * S],
                             np.full(npad - S, -1, np.int64)])
        perm = np.argsort(dc, kind="stable")
        perms.append(perm)
        inv = np.empty(npad, np.int64)
        inv[perm] = np.arange(npad)
        pos_of[c * S:(c + 1) * S] = inv[:S]
        degs_pad_c.append(np.maximum(dc[perm], 0))

    rows_k = [w * P for w in cfg["chunk_wins"]]
    chunk_base = np.concatenate(
        [[0], np.cumsum([NCORES * r for r in rows_k])]).astype(np.int64)
    node_core = np.arange(n, dtype=np.int64) // S
    k_of = np.minimum(pos_of // (cw * P), nchunk - 1)
    p2row = (chunk_base[k_of] + node_core * np.array(rows_k)[k_of]
             + (pos_of - k_of * cw * P)).astype(np.int64)

    # shared window max degree
    dwin = np.zeros(nwin, np.int64)
    for c in range(NCORES):
        dwin = np.maximum(dwin, degs_pad_c[c].reshape(nwin, P).max(axis=1))
    dwin = np.maximum(dwin, 1)

    # shared greedy groups within chunk boundaries
    groups = []
    for k in range(nchunk):
        w0k, w1k = k * cw, min((k + 1) * cw, nwin)
        w = w0k
        while w < w1k:
            gw, D = 1, int(dwin[w])
            for g2 in range(2, MAX_GW + 1):
                if w + g2 > w1k:
                    break
                d2 = int(dwin[w:w + g2].max())
                if g2 * P * d2 <= CAP_EDGES:
                    gw, D = g2, d2
                else:
                    break
            groups.append((w, gw, D, k))
            w += gw

    # stage pairs: two consecutive groups share one stage-1 buffer
    pairs = []
    gi = 0
    while gi < len(groups):
        ge = min(gi + 2, len(groups))
        pairs.append((gi, ge))
        gi = ge

    # per-core edge arrays sorted by (core, pos, src)
    pos_of_dst = pos_of[dst]
    order = np.lexsort((src, pos_of_dst, core_of))
    e_core, e_pos, e_src = core_of[order], pos_of_dst[order], src[order]
    core_sl = [slice(*np.searchsorted(e_core, [c, c + 1]))
               for c in range(NCORES)]

    def pair_span(pr):
        g0, ge = pr
        w0 = groups[g0][0]
        wl, gwl = groups[ge - 1][0], groups[ge - 1][1]
        return w0 * P, (wl + gwl) * P

    # per (core, pair): stage tables; per (core, group): stage-2 grids
    def stage_tables(bounds, nblk, zero_row, rowid_fn):
        """Returns shared call meta + per-core wrapped idx + per-core grids."""
        # counts per (group, core, block)
        cnt = np.zeros((len(pairs), NCORES, nblk), np.int64)
        blk_e = []   # per core: block id per edge
        rid_e = []   # per core: rowid per edge
        for c in range(NCORES):
            cpos = e_pos[core_sl[c]]
            rid = rowid_fn(c)
            blk = np.minimum(np.searchsorted(bounds, rid, side="right") - 1,
                             nblk - 1)
            blk_e.append(blk)
            rid_e.append(rid)
            for pi, pr in enumerate(pairs):
                p0s, p1s = pair_span(pr)
                lo, hi = np.searchsorted(cpos, [p0s, p1s])
                cnt[pi, c] += np.bincount(blk[lo:hi], minlength=nblk)
        cnt[:, :, nblk - 1] += 1          # zero entry
        tq = cnt.max(axis=1)              # [npairs, nblk]
        tpad = -(-tq // 128) * 128
        bases = np.concatenate(
            [np.zeros((len(pairs), 1), np.int64), np.cumsum(tpad, axis=1)],
            axis=1)
        nstage = bases[:, -1]
        assert nstage.max() < 32768
        calls = []   # per pair: list of (q, num_idxs, col_off16, b0, b1)
        coloff = 0
        for pi in range(len(pairs)):
            cl = []
            for q in range(nblk):
                if tpad[pi, q]:
                    cl.append((q, int(tpad[pi, q]), coloff,
                               bounds[q], bounds[q + 1]))
                    coloff += int(tpad[pi, q]) // 16
            calls.append(cl)
        total16 = coloff

        idx_cores = []
        grids = []
        for c in range(NCORES):
            cpos = e_pos[core_sl[c]]
            blk = blk_e[c]
            rid = rid_e[c]
            wr = np.full((16, total16), -1, np.int16)
            grid_list = []
            for pi, pr in enumerate(pairs):
                p0s, p1s = pair_span(pr)
                lo, hi = np.searchsorted(cpos, [p0s, p1s])
                gblk = blk[lo:hi]
                grid = rid[lo:hi]
                slots = np.zeros(hi - lo, np.int64)
                zero_slot = int(bases[pi, nblk - 1])
                for (q, tp, co16, b0, b1) in calls[pi]:
                    m = gblk == q
                    ids = grid[m] - b0
                    if q == nblk - 1:
                        ids = np.concatenate([[zero_row - b0], ids])
                    t = len(ids)
                    buf = np.zeros(tp, np.int64)
                    buf[:t] = ids
                    assert t <= tp and (t == 0 or (ids.max() < 32768
                                                   and ids.min() >= 0))
                    wr[:, co16:co16 + tp // 16] = _wrap16(buf)
                    base = int(bases[pi, q])
                    if q == nblk - 1:
                        slots[m] = base + 1 + np.arange(t - 1)
                    else:
                        slots[m] = base + np.arange(t)
                for gi in range(pr[0], pr[1]):
                    w0, gw, D, k = groups[gi]
                    Gn = gw * P
                    glo, ghi = np.searchsorted(cpos, [w0 * P, (w0 + gw) * P])
                    gpos = cpos[glo:ghi] - w0 * P
                    gslots = slots[glo - lo:ghi - lo]
                    jj = np.arange(ghi - glo) - np.concatenate(
                        [[0], np.cumsum(np.bincount(gpos, minlength=Gn))]
                    )[gpos]
                    gridm = np.full((Gn, D), zero_slot, np.int64)
                    gridm[gpos, jj] = gslots
                    assert gridm.max() < 32768
                    grid_list.append(_wrap16(gridm.ravel()))
            idx_cores.append(wr)
            grids.append(np.concatenate(grid_list, axis=1))
        grid_off = np.concatenate(
            [[0], np.cumsum([(g[1] * P * g[2]) // 16 for g in groups])])
        return dict(calls=calls, nstage=nstage, idx=idx_cores, grids=grids,
                    grid_off=grid_off, total16=total16)

    st1 = stage_tables(b1b, nblk1, xrows - 1,
                       lambda c: e_src[core_sl[c]])
    st2 = stage_tables(b2b, nblk2, p2_rows - 1,
                       lambda c: p2row[e_src[core_sl[c]]])

    cores = []
    for c in range(NCORES):
        perm = perms[c]
        real = perm < S
        xp = np.zeros((npad, IN_CH), np.float32)
        xp[real] = x[c * S + perm[real]]
        dinvp = np.zeros(npad, np.float32)
        dinvp[real] = deg_inv[c * S + perm[real]]
        cores.append(dict(xt=xp.T.copy(), dinv=dinvp.reshape(1, npad),
                          xsh=x[c * S:(c + 1) * S], perm=perm))

    shared = dict(groups=groups, pairs=pairs,
                  st1calls=st1["calls"], st1n=st1["nstage"],
                  st1w=st1["total16"], st2calls=st2["calls"],
                  st2n=st2["nstage"], st2w=st2["total16"],
                  grid_off=st1["grid_off"], chunk_base=chunk_base,
                  rows_k=rows_k)
    data = dict(cores=cores, g1=st1["idx"], g2=st1["grids"],
                h1=st2["idx"], h2=st2["grids"])
    return shared, data


# ---------------------------------------------------------------- builder

def _build(cfg, shared):
    import concourse.bacc as bacc
    import concourse.mybir as mybir
    import concourse.tile as tile

    f32 = mybir.dt.float32
    bf16 = mybir.dt.bfloat16
    i16 = mybir.dt.int16

    S, nwin, npad = cfg["shard"], cfg["nwin"], cfg["npad"]
    nchunk, cw = cfg["nchunk"], cfg["cw"]
    xrows, p2_rows = cfg["xrows"], cfg["p2_rows"]
    groups = shared["groups"]
    pairs = shared["pairs"]
    st1calls, st1n = shared["st1calls"], shared["st1n"]
    st2calls, st2n = shared["st2calls"], shared["st2n"]
    grid_off = shared["grid_off"]
    chunk_base, rows_k = shared["chunk_base"], shared["rows_k"]
    maxst = int(max(st1n.max(), st2n.max()))
    maxGnD = max(g[1] * P * g[2] for g in groups)
    maxGn = max(g[1] for g in groups) * P

    nc = bacc.Bacc()
    dp = nc.declare_dram_parameter
    xsh = dp("xsh", [S, IN_CH], bf16, isOutput=False)
    xtp = dp("xtp", [P, npad], bf16, isOutput=False)
    dinv = dp("dinv", [1, npad], f32, isOutput=False)
    g1p = dp("g1p", [16, shared["st1w"]], i16, isOutput=False)
    g2p = dp("g2p", [16, int(grid_off[-1])], i16, isOutput=False)
    h1p = dp("h1p", [16, shared["st2w"]], i16, isOutput=False)
    h2p = dp("h2p", [16, int(grid_off[-1])], i16, isOutput=False)
    w1l = dp("w1l", [IN_CH, HIDDEN], bf16, isOutput=False)
    w1r = dp("w1r", [IN_CH, HIDDEN], bf16, isOutput=False)
    w2l = dp("w2l", [HIDDEN, OUT_CH], bf16, isOutput=False)
    w2r = dp("w2r", [HIDDEN, OUT_CH], bf16, isOutput=False)
    b1c = dp("b1c", [P, 1], f32, isOutput=False)
    b2c = dp("b2c", [OUT_CH, 1], f32, isOutput=False)
    y = dp("y", [OUT_CH, npad], f32, isOutput=True)

    xdev = nc.dram_tensor("xdev", [xrows, IN_CH], bf16, addr_space="Shared")
    p2f = nc.dram_tensor("p2f", [p2_rows, P], bf16, addr_space="Shared")
    # 128-replicated idx tables (built on device once)
    g1r = nc.dram_tensor("g1r", [P, shared["st1w"]], i16)
    g2r = nc.dram_tensor("g2r", [P, int(grid_off[-1])], i16)
    h1r = nc.dram_tensor("h1r", [P, shared["st2w"]], i16)
    h2r = nc.dram_tensor("h2r", [P, int(grid_off[-1])], i16)

    with tile.TileContext(nc) as tc:
        with (
            tc.tile_pool(name="const", bufs=1) as cb,
            tc.tile_pool(name="st", bufs=1) as stp,
            tc.tile_pool(name="sl", bufs=2) as slp,
            tc.tile_pool(name="sb", bufs=2) as sb,
            tc.tile_pool(name="ix", bufs=3) as ixp,
            tc.tile_pool(name="psa", bufs=2, space="PSUM") as psa,
            tc.tile_pool(name="psb", bufs=2, space="PSUM") as psb,
            tc.tile_pool(name="psr", bufs=1, space="PSUM") as psr,
            tc.tile_pool(name="psd", bufs=2, space="PSUM") as psd,
            tc.tile_pool(name="dram", bufs=2, space="DRAM") as dr,
        ):
            def cload(param, shape, dt, tag):
                t = cb.tile(shape, dt, tag=tag)
                nc.sync.dma_start(out=t[:], in_=param[:])
                return t

            w1l_t = cload(w1l, [P, HIDDEN], bf16, "w1l")
            w1r_t = cload(w1r, [P, HIDDEN], bf16, "w1r")
            w2l_t = cload(w2l, [P, OUT_CH], bf16, "w2l")
            w2r_t = cload(w2r, [P, OUT_CH], bf16, "w2r")
            b1_t = cload(b1c, [P, 1], f32, "b1")
            b2_t = cload(b2c, [OUT_CH, 1], f32, "b2")
            xt_t = cload(xtp, [P, npad], bf16, "xt")
            zrow = cb.tile([P, P], bf16, tag="zrow")
            nc.vector.memset(zrow[:], 0.0)
            ones_t = cb.tile([1, P], f32, tag="ones")
            nc.vector.memset(ones_t[:], 1.0)
            r2_t = cb.tile([OUT_CH, npad], f32, tag="r2")

            # replicate idx tables [16, W] -> [128, W] in DRAM
            for par, rep in ((g1p, g1r), (g2p, g2r), (h1p, h1r), (h2p, h2r)):
                for rr in range(8):
                    nc.sync.dma_start(out=rep[rr * 16:(rr + 1) * 16, :],
                                      in_=par[:])

            # zero rows
            nc.sync.dma_start(out=xdev[xrows - 1:xrows, :], in_=zrow[:1, :])
            nc.sync.dma_start(out=p2f[p2_rows - 1:p2_rows, :], in_=zrow[:1, :])

            # x AllGather
            xc = dr.tile([S, IN_CH], bf16, tag="xc")
            nc.sync.dma_start(out=xc[:], in_=xsh[:])
            nc.gpsimd.collective_compute(
                "AllGather", mybir.AluOpType.bypass,
                replica_groups=[list(range(NCORES))],
                ins=[xc.opt()],
                outs=[xdev[0:NCORES * S, :]],
            )

            p2c = []
            for k in range(nchunk):
                p2ck = dr.tile([rows_k[k], P], bf16, tag=f"p2c{k}")
                p2c.append(p2ck)

            def do_stage(gi, calls, nst, idx_rep, table_ap, tag):
                """stage-1 gathers + DMA to DRAM staging; returns dram tile."""
                n = int(nst[gi])
                stt = stp.tile([P, (maxst // P) * IN_CH], bf16, tag="st")
                c0 = calls[gi][0][2]
                cn = sum(num for (_, num, _, _, _) in calls[gi]) // 16
                it = ixp.tile([P, maxst // 16], i16, tag="it")
                nc.sync.dma_start(out=it[:, 0:cn],
                                  in_=idx_rep[:, c0:c0 + cn])
                base = 0
                for (q, num, co16, b0, b1_) in calls[gi]:
                    nc.gpsimd.dma_gather(
                        out_ap=stt[:, (base // P) * IN_CH:
                                   ((base + num) // P) * IN_CH]
                        .rearrange("p (c e) -> p c e", e=IN_CH),
                        in_ap=table_ap(b0, b1_),
                        idxs_ap=it[:, co16 - c0:co16 - c0 + num // 16],
                        num_idxs=num, num_idxs_reg=num,
                        elem_size=IN_CH, single_packet=False)
                    base += num
                sd = dr.tile([maxst, IN_CH], bf16, tag="sd")
                nc.sync.dma_start(
                    out=sd[0:n, :].rearrange("(c p) e -> p c e", p=P),
                    in_=stt[:, 0:(n // P) * IN_CH].rearrange(
                        "p (c e) -> p c e", e=IN_CH))
                return sd, n

            def do_slab(gi, sd, n, grid_rep, tag):
                w0, gw, D, k = groups[gi]
                Gn = gw * P
                it = ixp.tile([P, (Gn * D) // 16], i16, tag="gg")
                nc.sync.dma_start(
                    out=it[:],
                    in_=grid_rep[:, int(grid_off[gi]):int(grid_off[gi + 1])])
                slab = slp.tile([P, maxGnD], bf16, tag="sl")
                nc.gpsimd.dma_gather(
                    out_ap=slab[:, 0:Gn * D].rearrange(
                        "p (c e) -> p c e", c=1),
                    in_ap=sd[0:n, :],
                    idxs_ap=it[:],
                    num_idxs=Gn * D, num_idxs_reg=Gn * D,
                    elem_size=IN_CH, transpose=True, single_packet=False)
                return slab

            # ---------------- phase A ----------------
            for pi, pr in enumerate(pairs):
              sdp, n1p = do_stage(pi, st1calls, st1n, g1r,
                                  lambda b0, b1_: xdev[b0:b1_, :], "1")
              for gi in range(pr[0], pr[1]):
                w0, gw, D, k = groups[gi]
                Gn = gw * P
                p0 = w0 * P
                slab = do_slab(gi, sdp, n1p, g2r, "1")
                dv = ixp.tile([1, maxGn], f32, tag="dv")
                nc.sync.dma_start(out=dv[0:1, 0:Gn],
                                  in_=dinv[0:1, p0:p0 + Gn])
                agg = sb.tile([P, maxGn], f32, tag="agg")
                nc.vector.tensor_reduce(
                    out=agg[:, 0:Gn],
                    in_=slab[:, 0:Gn * D].rearrange("p (n d) -> p n d", d=D),
                    axis=mybir.AxisListType.X, op=mybir.AluOpType.add)
                psdv = psd.tile([P, maxGn], f32, tag="psdv", space="PSUM")
                nc.tensor.matmul(out=psdv[:, 0:Gn], lhsT=ones_t[0:1, :],
                                 rhs=dv[0:1, 0:Gn], start=True, stop=True)
                aggs = sb.tile([P, maxGn], bf16, tag="aggs")
                nc.vector.tensor_tensor(
                    out=aggs[:, 0:Gn], in0=agg[:, 0:Gn],
                    in1=psdv[:, 0:Gn], op=mybir.AluOpType.mult)
                ps1 = psa.tile([P, maxGn], f32, tag="ps1", space="PSUM")
                nc.tensor.matmul(out=ps1[:, 0:Gn], lhsT=w1l_t[:],
                                 rhs=aggs[:, 0:Gn], start=True, stop=False)
                nc.tensor.matmul(out=ps1[:, 0:Gn], lhsT=w1r_t[:],
                                 rhs=xt_t[:, p0:p0 + Gn], start=False,
                                 stop=True)
                h4 = sb.tile([P, maxGn], bf16, tag="h4")
                nc.vector.tensor_scalar(
                    out=h4[:, 0:Gn], in0=ps1[:, 0:Gn], scalar1=b1_t[:, 0:1],
                    scalar2=0.0, op0=mybir.AluOpType.add,
                    op1=mybir.AluOpType.max)
                h4c = sb.tile([P, maxGn], bf16, tag="h4c")
                nc.vector.tensor_copy(out=h4c[:, 0:Gn], in_=h4[:, 0:Gn])
                # P2 rows (node-major) for this group's windows
                ps2 = psb.tile([P, MAX_GW * OUT_CH], f32, tag="ps2",
                               space="PSUM")
                for wi in range(gw):
                    nc.tensor.matmul(
                        out=ps2[:, wi * OUT_CH:(wi + 1) * OUT_CH],
                        lhsT=h4c[:, wi * P:(wi + 1) * P], rhs=w2l_t[:],
                        start=True, stop=True)
                p2sb = sb.tile([P, MAX_GW * P], bf16, tag="p2sb")
                nc.vector.memset(p2sb[:, 0:gw * P], 0.0)
                nc.vector.tensor_copy(
                    out=p2sb[:, 0:gw * P].rearrange(
                        "p (w c) -> p w c", c=P)[:, :, 0:OUT_CH],
                    in_=ps2[:, 0:gw * OUT_CH].rearrange(
                        "p (w c) -> p w c", c=OUT_CH))
                q0 = (w0 - k * cw) * P
                nc.sync.dma_start(
                    out=p2c[k][q0:q0 + Gn, :].rearrange(
                        "(w l) c -> l w c", l=P),
                    in_=p2sb[:, 0:gw * P].rearrange(
                        "l (w c) -> l w c", c=P))
                # R2 = h @ W2r + b2 (transposed), kept in SBUF
                ps3 = psr.tile([OUT_CH, maxGn], f32, tag="ps3", space="PSUM")
                nc.tensor.matmul(out=ps3[:, 0:Gn], lhsT=w2r_t[:],
                                 rhs=h4[:, 0:Gn], start=True, stop=True)
                nc.vector.tensor_scalar(
                    out=r2_t[:, p0:p0 + Gn], in0=ps3[:, 0:Gn],
                    scalar1=b2_t[:, 0:1], scalar2=None,
                    op0=mybir.AluOpType.add)

                if gi + 1 == len(groups) or groups[gi + 1][3] != k:
                    nc.gpsimd.collective_compute(
                        "AllGather", mybir.AluOpType.bypass,
                        replica_groups=[list(range(NCORES))],
                        ins=[p2c[k].opt()],
                        outs=[p2f[int(chunk_base[k]):int(chunk_base[k + 1]),
                                  :]],
                    )

            # ---------------- phase B ----------------
            for pi, pr in enumerate(pairs):
              sdp, n2p = do_stage(pi, st2calls, st2n, h1r,
                                  lambda b0, b1_: p2f[b0:b1_, :], "2")
              for gi in range(pr[0], pr[1]):
                w0, gw, D, k = groups[gi]
                Gn = gw * P
                p0 = w0 * P
                slab = do_slab(gi, sdp, n2p, h2r, "2")
                dv = ixp.tile([1, maxGn], f32, tag="dv")
                nc.sync.dma_start(out=dv[0:1, 0:Gn],
                                  in_=dinv[0:1, p0:p0 + Gn])
                agg = sb.tile([P, maxGn], f32, tag="agg")
                nc.vector.tensor_reduce(
                    out=agg[:, 0:Gn],
                    in_=slab[:, 0:Gn * D].rearrange("p (n d) -> p n d", d=D),
                    axis=mybir.AxisListType.X, op=mybir.AluOpType.add)
                psdv = psd.tile([P, maxGn], f32, tag="psdv", space="PSUM")
                nc.tensor.matmul(out=psdv[:, 0:Gn], lhsT=ones_t[0:1, :],
                                 rhs=dv[0:1, 0:Gn], start=True, stop=True)
                ym = sb.tile([OUT_CH, maxGn], f32, tag="ym")
                nc.vector.tensor_tensor(
                    out=ym[:, 0:Gn], in0=agg[0:OUT_CH, 0:Gn],
                    in1=psdv[0:OUT_CH, 0:Gn], op=mybir.AluOpType.mult)
                yo = sb.tile([OUT_CH, maxGn], f32, tag="yo")
                nc.vector.tensor_tensor(
                    out=yo[:, 0:Gn], in0=ym[:, 0:Gn],
                    in1=r2_t[:, p0:p0 + Gn], op=mybir.AluOpType.add)
                nc.sync.dma_start(out=y[:, p0:p0 + Gn], in_=yo[:, 0:Gn])

    nc.compile()
    return nc


# ---------------------------------------------------------------- entry

_CACHE = {}


def _to_bf16(a):
    import ml_dtypes
    return np.asarray(a, np.float32).astype(ml_dtypes.bfloat16)


def _make_inmaps(inputs, shared, data):
    common = dict(
        w1l=_to_bf16(inputs["W1_l"]),
        w1r=_to_bf16(inputs["W1_r"]),
        w2l=_to_bf16(inputs["W2_l"]),
        w2r=_to_bf16(inputs["W2_r"]),
        b1c=np.asarray(inputs["b1"], np.float32).reshape(P, 1),
        b2c=np.asarray(inputs["b2"], np.float32).reshape(OUT_CH, 1),
    )
    in_maps = []
    for c in range(NCORES):
        d = data["cores"][c]
        m = dict(common)
        m["xsh"] = _to_bf16(d["xsh"])
        m["xtp"] = _to_bf16(d["xt"])
        m["dinv"] = d["dinv"]
        m["g1p"] = data["g1"][c]
        m["g2p"] = data["g2"][c]
        m["h1p"] = data["h1"][c]
        m["h2p"] = data["h2"][c]
        in_maps.append(m)
    return in_maps


def kernel(x, edge_index, W1_l, W1_r, b1, W2_l, W2_r, b2):
    x = np.asarray(x, dtype=np.float32)
    edge_index = np.asarray(edge_index)
    cfg = _derive_cfg(x.shape[0])
    shared, data = _preprocess(x, edge_index, cfg)

    key = (x.shape, edge_index.shape)
    sig = _sig_of(shared)
    if key in _CACHE and _CACHE[key][1] == sig:
        nc = _CACHE[key][0]
    else:
        nc = _build(cfg, shared)
        _CACHE[key] = (nc, sig)

    in_maps = _make_inmaps(
        dict(W1_l=W1_l, W1_r=W1_r, b1=b1, W2_l=W2_l, W2_r=W2_r, b2=b2),
        shared, data)

    from concourse.bass_utils import run_bass_kernel_spmd
    r = run_bass_kernel_spmd(nc, in_maps, core_ids=list(range(NCORES)))
    S = cfg["shard"]
    out = np.zeros((x.shape[0], OUT_CH), np.float32)
    for c in range(NCORES):
        yc = np.asarray(r.results[c]["y"], np.float32).T  # [npad, 64]
        perm = data["cores"][c]["perm"]
        real = perm < S
        out[c * S + perm[real]] = yc[real]
    return np.ascontiguousarray(out, dtype=np.float32)


def _sig_of(shared):
    return (tuple(shared["groups"]), tuple(shared["pairs"]),
            int(shared["st1w"]),
            int(shared["st2w"]), int(shared["grid_off"][-1]))
